# revision 27
# baseline (speedup 1.0000x reference)
"""MoE feed-forward (E=8 experts, top-2) for one TRN2 chip (8 NeuronCores).

Strategy: expert-parallel with capacity-1.0 dispatch. Host computes the
(tiny) router matmul + softmax + top-2 in numpy, gathers each expert's
routed tokens up to a fixed capacity C = 1024 (= the mean expert load
T*TOPK/E, so every core streams identical, perfectly balanced work in two
512-wide PSUM chunks), and ships per-expert weights + gathered tokens to
one core each. The small routing overflow beyond capacity (~1% of
token-expert pairs at this routing) is FFN'd on host in f32. Each core
runs an identical Bass/Tile FFN program in bf16:

    GT = Wg^T @ X   (transposed-activation layout: [I, C] tiles)
    UT = Wu^T @ X
    AT = silu(GT) * UT          (bf16, SBUF-resident)
    YT = Wd^T_col-tiles @ AT    -> [H, C] fp16 out

All matmul operands use natural (row-major chunk) layouts, so no on-device
transposes are needed. Gate/up weights are DMA'd in per-i-tile column blocks
so arrival order matches the PE's consumption order (the first i-tile needs
only 512KB of weights, not all 8MB). The host applies the top-2 combine
weights and scatters rows back into the full [B, S, H] output.

The PE matmul stream is the kernel: 768 warm 512-wide matmuls x 216ns =
166us of the ~186us exec. The rest is launch preamble (~6.5us), the
mandatory HAM warmup (~4.3us, see comment in the program), and the
end-of-program drain/barrier (~4us).
"""

import numpy as np
import ml_dtypes

H = 1024
I = 2048
E = 8
TOPK = 2
P = 128
N_T = 2  # token chunks per core (chunk width C/2 <= 512 = one PSUM bank)
C_CAP = 1024  # expert capacity: tokens beyond this are FFN'd on host (<=2%)

_PROGRAM_CACHE = {}
LAST_RESULT = None  # BassKernelResults of the most recent device run


def _build_program(C):
    from contextlib import ExitStack

    import concourse.mybir as mybir
    import concourse.tile as tile
    from concourse import bacc

    f32 = mybir.dt.float32
    bf16 = mybir.dt.bfloat16
    Silu = mybir.ActivationFunctionType.Silu

    n_h = H // P   # 8 contraction chunks over hidden dim
    n_i = I // P   # 16 tiles over intermediate dim
    NT = C // N_T  # token-chunk width
    assert C % N_T == 0 and NT <= 512

    nc = bacc.Bacc("TRN2", enable_partition_id=False)
    xT = nc.dram_tensor("xT", [H, C], bf16, kind="ExternalInput")
    # gate/up are host-prearranged to [p, i_tile, c, j] so each per-i-tile
    # DMA reads 2KB-contiguous lines per partition (full DMA rate).
    wg = nc.dram_tensor("wg", [P, I // P, H // P, P], bf16, kind="ExternalInput")
    wu = nc.dram_tensor("wu", [P, I // P, H // P, P], bf16, kind="ExternalInput")
    wd = nc.dram_tensor("wd", [I, H], bf16, kind="ExternalInput")
    f16 = mybir.dt.float16
    yT = nc.dram_tensor("yT", [H, C], f16, kind="ExternalOutput")

    with tile.TileContext(nc) as tc:
        with ExitStack() as ctx:
            wpool = ctx.enter_context(tc.tile_pool(name="weights", bufs=1))
            atpool = ctx.enter_context(tc.tile_pool(name="atp", bufs=1))
            spool = ctx.enter_context(tc.tile_pool(name="stmp", bufs=4))
            ypool = ctx.enter_context(tc.tile_pool(name="yst", bufs=4))
            pspool = ctx.enter_context(
                tc.tile_pool(name="ps", bufs=8, space="PSUM")
            )

            # HAM warmup — REQUIRED, and it must fill a complete ~3.4us
            # activity window: these LDWEIGHTS-free back-to-back matmuls are
            # what releases the PE clock throttle (4/8 -> 8/8). Without them
            # the real stream (with per-MM weight loads) never un-throttles
            # and every matmul runs ~20% slow for the whole kernel.
            # 36 x ~107ns = 3.85us: one full HAM window plus margin (the
            # window is free-running, so shorter warmups lose a per-core
            # phase lottery and the first real MMs run cold), and it also
            # covers the ~3.5us dual-queue input-DMA head (x0+wg0+wu0,
            # 8 cores sharing HBM) so the real stream starts warm, no gap.
            warm_src = wpool.tile([P, P], bf16, name="warm_src")
            nc.vector.memset(warm_src, 0.0)
            warm_ps = pspool.tile([P, NT], f32, tag="ps", name="warm_ps")
            for _ in range(36):
                nc.tensor.matmul(
                    warm_ps[:, 0:P], warm_src, warm_src, start=True, stop=True
                )

            x_s = wpool.tile([P, n_h, C], bf16, name="x_s")
            # gate/up keyed by i-tile: [p, i_tile, c, i_within]
            wg_s = wpool.tile([P, n_i, n_h, P], bf16, name="wg_s")
            wu_s = wpool.tile([P, n_i, n_h, P], bf16, name="wu_s")
            wd_s = wpool.tile([P, n_i, H], bf16, name="wd_s")
            at_s = atpool.tile([P, n_i, C], bf16, name="at_s")

            # Big-block loads in consumption order on the SP HWDGE ring
            # (FIFO order IS the priority order): x chunk 0 and i-tile 0's
            # gate/up gate the first real matmuls; the rest of x during
            # i-tile 0's sweep; remaining gate/up just-in-time; then wd
            # (phase B only needs it ~120us in). Full 2KB-per-partition
            # lines run the DMA engines at full rate — splitting the head
            # into small "critical-path-first" pieces measures ~4us WORSE
            # (short lines + extra descriptors slow the first i-tile).
            # Dual HWDGE issue: each dma_start costs ~600ns of serial issue
            # on its DGE ring, and sync + scalar both have HWDGE rings. The
            # head (x0+wg0+wu0 gates the first real matmul) issues x0/wg0 on
            # sync concurrently with wu0 on scalar, and the 16 wd loads
            # (needed only in phase B) ride scalar's ring so they never pace
            # sync's just-in-time gate/up stream.
            nc.sync.dma_start(out=x_s[:, 0, :], in_=xT[0:P, :])
            nc.scalar.dma_start(out=wu_s[:, 0, :, :], in_=wu[:, 0, :, :])
            nc.sync.dma_start(out=wg_s[:, 0, :, :], in_=wg[:, 0, :, :])
            for c in range(1, n_h):
                nc.sync.dma_start(
                    out=x_s[:, c, :], in_=xT[c * P:(c + 1) * P, :]
                )
            for it in range(1, n_i):
                nc.sync.dma_start(out=wg_s[:, it, :, :], in_=wg[:, it, :, :])
                nc.sync.dma_start(out=wu_s[:, it, :, :], in_=wu[:, it, :, :])
            for it in range(n_i):
                nc.scalar.dma_start(
                    out=wd_s[:, it, :], in_=wd[it * P:(it + 1) * P, :]
                )

            # Phase A: AT[i_tile, tok] = silu(Wg^T X) * (Wu^T X)
            for it in range(n_i):
                g_ps = [
                    pspool.tile([P, NT], f32, tag="ps", name=f"g_{it}_{k}")
                    for k in range(N_T)
                ]
                u_ps = [
                    pspool.tile([P, NT], f32, tag="ps", name=f"u_{it}_{k}")
                    for k in range(N_T)
                ]
                for c in range(n_h):
                    lg = wg_s[:, it, c, :]
                    lu = wu_s[:, it, c, :]
                    st, sp = (c == 0), (c == n_h - 1)
                    for k in range(N_T):
                        nc.tensor.matmul(
                            g_ps[k], lg, x_s[:, c, k * NT:(k + 1) * NT],
                            start=st, stop=sp,
                        )
                    for k in range(N_T):
                        nc.tensor.matmul(
                            u_ps[k], lu, x_s[:, c, k * NT:(k + 1) * NT],
                            start=st, stop=sp,
                        )
                for k in range(N_T):
                    stile = spool.tile([P, NT], f32, tag="stmp", name=f"s_{it}_{k}")
                    nc.scalar.activation(stile, g_ps[k], Silu)
                    nc.vector.tensor_mul(
                        at_s[:, it, k * NT:(k + 1) * NT], stile, u_ps[k]
                    )

            # Phase B: YT[h_tile, tok] = sum_i Wd[i, h_tile]^T AT[i, tok]
            # k is the OUTER loop so chunk k's full contraction finishes
            # before chunk k+1 starts: its cast+DMA-out overlap k+1's MMs,
            # leaving only one [P, NT] cast+DMA as the post-stream drain.
            for ht in range(n_h):
                for k in range(N_T):
                    y_ps = pspool.tile([P, NT], f32, tag="ps", name=f"y_{ht}_{k}")
                    for it in range(n_i):
                        ld = wd_s[:, it, ht * P:(ht + 1) * P]
                        nc.tensor.matmul(
                            y_ps, ld, at_s[:, it, k * NT:(k + 1) * NT],
                            start=(it == 0), stop=(it == n_i - 1),
                        )
                    # One cast + one DMA per group: each dma_start costs a
                    # ~600ns serial DIRECT2D on the sync queue, so splitting
                    # the final group into quarters measures ~1us WORSE than
                    # a single [P, NT] cast (691ns) + one DMA.
                    yt = ypool.tile([P, NT], f16, tag="yst", name=f"yo_{ht}_{k}")
                    nc.vector.tensor_copy(yt, y_ps)
                    nc.sync.dma_start(
                        out=yT[ht * P:(ht + 1) * P, k * NT:(k + 1) * NT],
                        in_=yt,
                    )

    nc.compile()
    return nc


def kernel(x, gate_w, wg, wu, wd):
    global LAST_RESULT
    x = np.asarray(x, dtype=np.float32)
    gate_w = np.asarray(gate_w, dtype=np.float32)
    wg = np.asarray(wg, dtype=np.float32)
    wu = np.asarray(wu, dtype=np.float32)
    wd = np.asarray(wd, dtype=np.float32)

    B, S, Hh = x.shape
    T = B * S
    xf = np.ascontiguousarray(x.reshape(T, Hh))

    # Router (tiny): logits -> softmax -> top-2, matching jax.lax.top_k
    # tie-order (stable sort prefers the lower expert index).
    logits = xf @ gate_w.T
    logits -= logits.max(axis=-1, keepdims=True)
    np.exp(logits, out=logits)
    probs = logits / logits.sum(axis=-1, keepdims=True)
    order = np.argsort(-probs, axis=1, kind="stable")[:, :TOPK]

    onehot = np.zeros((T, E), dtype=bool)
    onehot[np.arange(T)[:, None], order] = True
    full_lists = [np.nonzero(onehot[:, e])[0] for e in range(E)]
    # Capacity dispatch: each core FFNs at most C_CAP tokens for its expert;
    # the small overflow (~1% of token-expert pairs at this routing) is
    # FFN'd on host in f32 below.
    tok_lists = [t[:C_CAP] for t in full_lists]
    over_lists = [t[C_CAP:] for t in full_lists]
    maxc = max(max(len(t) for t in tok_lists), N_T)
    C = int(-(-maxc // N_T) * N_T)  # round up to a multiple of N_T
    assert C <= C_CAP and C % N_T == 0 and C // N_T <= 512

    nc = _PROGRAM_CACHE.get(C)
    if nc is None:
        nc = _build_program(C)
        _PROGRAM_CACHE[C] = nc

    bf = ml_dtypes.bfloat16
    xf_bf = xf.astype(bf)

    def _gu_layout(w):  # [H, I] -> [p, i_tile, c, j]
        return np.ascontiguousarray(
            w.reshape(H // P, P, I // P, P).transpose(1, 2, 0, 3)
        )

    in_maps = []
    for e in range(E):
        idx = tok_lists[e]
        xe = np.zeros((C, Hh), dtype=bf)
        xe[: len(idx)] = xf_bf[idx]
        in_maps.append(
            {
                "xT": np.ascontiguousarray(xe.T),
                "wg": _gu_layout(wg[e].astype(bf)),
                "wu": _gu_layout(wu[e].astype(bf)),
                "wd": wd[e].astype(bf),
            }
        )

    from concourse.bass_utils import run_bass_kernel_spmd

    res = run_bass_kernel_spmd(nc, in_maps, core_ids=list(range(E)))
    LAST_RESULT = res

    out = np.zeros((T, Hh), dtype=np.float32)
    for e in range(E):
        idx = tok_lists[e]
        ye = np.asarray(res.results[e]["yT"]).T[: len(idx)]
        out[idx] += probs[idx, e][:, None] * ye.astype(np.float32)

    def _silu(v):
        return v / (1.0 + np.exp(-v))

    for e in range(E):
        idx = over_lists[e]
        if len(idx) == 0:
            continue
        xo = xf[idx]
        yo = (_silu(xo @ wg[e]) * (xo @ wu[e])) @ wd[e]
        out[idx] += probs[idx, e][:, None] * yo
    return out.reshape(B, S, Hh)



# revision 29
# speedup vs baseline: 1.0532x; 1.0532x over previous
"""MoE feed-forward (E=8 experts, top-2) for one TRN2 chip (8 NeuronCores).

Strategy: expert-parallel with capacity-1.0 dispatch. Host computes the
(tiny) router matmul + softmax + top-2 in numpy, gathers each expert's
routed tokens up to a fixed capacity C = 1024 (= the mean expert load
T*TOPK/E, so every core streams identical, perfectly balanced work in two
512-wide PSUM chunks), and ships per-expert weights + gathered tokens to
one core each. The small routing overflow beyond capacity (~1% of
token-expert pairs at this routing) is FFN'd on host in f32. Each core
runs an identical Bass/Tile FFN program in bf16:

    GT = Wg^T @ X   (transposed-activation layout: [I, C] tiles)
    UT = Wu^T @ X
    AT = silu(GT) * UT          (bf16, SBUF-resident)
    YT = Wd^T_col-tiles @ AT    -> [H, C] fp16 out

All matmul operands use natural (row-major chunk) layouts, so no on-device
transposes are needed. Gate/up weights are DMA'd in per-i-tile column blocks
so arrival order matches the PE's consumption order (the first i-tile needs
only 512KB of weights, not all 8MB). The host applies the top-2 combine
weights and scatters rows back into the full [B, S, H] output.

The PE matmul stream is the kernel: 768 warm 512-wide matmuls x 216ns =
166us of the ~186us exec. The rest is launch preamble (~6.5us), the
mandatory HAM warmup (~4.3us, see comment in the program), and the
end-of-program drain/barrier (~4us).
"""

import numpy as np
import ml_dtypes

H = 1024
I = 2048
E = 8
TOPK = 2
P = 128
N_T = 2  # token chunks per core (chunk width C/2 <= 512 = one PSUM bank)
C_CAP = 1024  # expert capacity: tokens beyond this are FFN'd on host (<=2%)

_PROGRAM_CACHE = {}
LAST_RESULT = None  # BassKernelResults of the most recent device run


def _build_program(C):
    from contextlib import ExitStack

    import concourse.mybir as mybir
    import concourse.tile as tile
    from concourse import bacc

    f32 = mybir.dt.float32
    bf16 = mybir.dt.bfloat16
    Silu = mybir.ActivationFunctionType.Silu

    n_h = H // P   # 8 contraction chunks over hidden dim
    n_i = I // P   # 16 tiles over intermediate dim
    NT = C // N_T  # token-chunk width
    assert C % N_T == 0 and NT <= 512

    nc = bacc.Bacc("TRN2", enable_partition_id=False)
    xT = nc.dram_tensor("xT", [H, C], bf16, kind="ExternalInput")
    # gate/up are host-prearranged to [p, i_tile, c, j] so each per-i-tile
    # DMA reads 2KB-contiguous lines per partition (full DMA rate).
    wg = nc.dram_tensor("wg", [P, I // P, H // P, P], bf16, kind="ExternalInput")
    wu = nc.dram_tensor("wu", [P, I // P, H // P, P], bf16, kind="ExternalInput")
    wd = nc.dram_tensor("wd", [I, H], bf16, kind="ExternalInput")
    f16 = mybir.dt.float16
    yT = nc.dram_tensor("yT", [H, C], f16, kind="ExternalOutput")

    with tile.TileContext(nc) as tc:
        with ExitStack() as ctx:
            wpool = ctx.enter_context(tc.tile_pool(name="weights", bufs=1))
            atpool = ctx.enter_context(tc.tile_pool(name="atp", bufs=1))
            spool = ctx.enter_context(tc.tile_pool(name="stmp", bufs=4))
            ypool = ctx.enter_context(tc.tile_pool(name="yst", bufs=4))
            pspool = ctx.enter_context(
                tc.tile_pool(name="ps", bufs=8, space="PSUM")
            )

            # HAM warmup — REQUIRED, and it must fill a complete ~3.4us
            # activity window: these LDWEIGHTS-free back-to-back matmuls are
            # what releases the PE clock throttle (4/8 -> 8/8). Without them
            # the real stream (with per-MM weight loads) never un-throttles
            # and every matmul runs ~20% slow for the whole kernel.
            # 40 x ~107ns = 4.3us: one full HAM window plus margin (the
            # window is free-running, so shorter warmups lose a per-core
            # phase lottery and the first real MMs run cold), and it also
            # covers the ~4us input-DMA head (x0+wg0+wu0, 8 cores sharing
            # HBM) so the real stream starts warm with no gap.
            warm_src = wpool.tile([P, P], bf16, name="warm_src")
            nc.vector.memset(warm_src, 0.0)
            warm_ps = pspool.tile([P, NT], f32, tag="ps", name="warm_ps")
            for _ in range(40):
                nc.tensor.matmul(
                    warm_ps[:, 0:P], warm_src, warm_src, start=True, stop=True
                )

            x_s = wpool.tile([P, n_h, C], bf16, name="x_s")
            # gate/up keyed by i-tile: [p, i_tile, c, i_within]
            wg_s = wpool.tile([P, n_i, n_h, P], bf16, name="wg_s")
            wu_s = wpool.tile([P, n_i, n_h, P], bf16, name="wu_s")
            wd_s = wpool.tile([P, n_i, H], bf16, name="wd_s")
            at_s = atpool.tile([P, n_i, C], bf16, name="at_s")

            # Big-block loads in consumption order on the SP HWDGE ring
            # (FIFO order IS the priority order): x chunk 0 and i-tile 0's
            # gate/up gate the first real matmuls; the rest of x during
            # i-tile 0's sweep; remaining gate/up just-in-time; then wd
            # (phase B only needs it ~120us in). Full 2KB-per-partition
            # lines run the DMA engines at full rate — splitting the head
            # into small "critical-path-first" pieces measures ~4us WORSE
            # (short lines + extra descriptors slow the first i-tile).
            # All loads on the single sync HWDGE ring, in consumption order:
            # the FIFO order IS the priority order, which is load-bearing —
            # x0+wg0+wu0 gate the first real matmuls, the rest of gate/up
            # streams just-in-time, and wd (only needed in phase B ~115us
            # in) transfers strictly last. Splitting across the scalar HWDGE
            # ring measures 4-8us WORSE: the wd transfers start early and
            # flood the shared DMA engines during the gate/up JIT window.
            nc.sync.dma_start(out=x_s[:, 0, :], in_=xT[0:P, :])
            nc.sync.dma_start(out=wg_s[:, 0, :, :], in_=wg[:, 0, :, :])
            nc.sync.dma_start(out=wu_s[:, 0, :, :], in_=wu[:, 0, :, :])
            for c in range(1, n_h):
                nc.sync.dma_start(
                    out=x_s[:, c, :], in_=xT[c * P:(c + 1) * P, :]
                )
            for it in range(1, n_i):
                nc.sync.dma_start(out=wg_s[:, it, :, :], in_=wg[:, it, :, :])
                nc.sync.dma_start(out=wu_s[:, it, :, :], in_=wu[:, it, :, :])
            for it in range(n_i):
                nc.sync.dma_start(
                    out=wd_s[:, it, :], in_=wd[it * P:(it + 1) * P, :]
                )

            # Phase A: AT[i_tile, tok] = silu(Wg^T X) * (Wu^T X)
            for it in range(n_i):
                g_ps = [
                    pspool.tile([P, NT], f32, tag="ps", name=f"g_{it}_{k}")
                    for k in range(N_T)
                ]
                u_ps = [
                    pspool.tile([P, NT], f32, tag="ps", name=f"u_{it}_{k}")
                    for k in range(N_T)
                ]
                for c in range(n_h):
                    lg = wg_s[:, it, c, :]
                    lu = wu_s[:, it, c, :]
                    st, sp = (c == 0), (c == n_h - 1)
                    for k in range(N_T):
                        nc.tensor.matmul(
                            g_ps[k], lg, x_s[:, c, k * NT:(k + 1) * NT],
                            start=st, stop=sp,
                        )
                    for k in range(N_T):
                        nc.tensor.matmul(
                            u_ps[k], lu, x_s[:, c, k * NT:(k + 1) * NT],
                            start=st, stop=sp,
                        )
                for k in range(N_T):
                    stile = spool.tile([P, NT], f32, tag="stmp", name=f"s_{it}_{k}")
                    nc.scalar.activation(stile, g_ps[k], Silu)
                    nc.vector.tensor_mul(
                        at_s[:, it, k * NT:(k + 1) * NT], stile, u_ps[k]
                    )

            # Phase B: YT[h_tile, tok] = sum_i Wd[i, h_tile]^T AT[i, tok]
            # k is the OUTER loop so chunk k's full contraction finishes
            # before chunk k+1 starts: its cast+DMA-out overlap k+1's MMs,
            # leaving only one [P, NT] cast+DMA as the post-stream drain.
            for ht in range(n_h):
                for k in range(N_T):
                    y_ps = pspool.tile([P, NT], f32, tag="ps", name=f"y_{ht}_{k}")
                    for it in range(n_i):
                        ld = wd_s[:, it, ht * P:(ht + 1) * P]
                        nc.tensor.matmul(
                            y_ps, ld, at_s[:, it, k * NT:(k + 1) * NT],
                            start=(it == 0), stop=(it == n_i - 1),
                        )
                    # One cast + one DMA per group: each dma_start costs a
                    # ~600ns serial DIRECT2D on the sync queue, so splitting
                    # the final group into quarters measures ~1us WORSE than
                    # a single [P, NT] cast (691ns) + one DMA.
                    yt = ypool.tile([P, NT], f16, tag="yst", name=f"yo_{ht}_{k}")
                    nc.vector.tensor_copy(yt, y_ps)
                    nc.sync.dma_start(
                        out=yT[ht * P:(ht + 1) * P, k * NT:(k + 1) * NT],
                        in_=yt,
                    )

    nc.compile()
    return nc


def kernel(x, gate_w, wg, wu, wd):
    global LAST_RESULT
    x = np.asarray(x, dtype=np.float32)
    gate_w = np.asarray(gate_w, dtype=np.float32)
    wg = np.asarray(wg, dtype=np.float32)
    wu = np.asarray(wu, dtype=np.float32)
    wd = np.asarray(wd, dtype=np.float32)

    B, S, Hh = x.shape
    T = B * S
    xf = np.ascontiguousarray(x.reshape(T, Hh))

    # Router (tiny): logits -> softmax -> top-2, matching jax.lax.top_k
    # tie-order (stable sort prefers the lower expert index).
    logits = xf @ gate_w.T
    logits -= logits.max(axis=-1, keepdims=True)
    np.exp(logits, out=logits)
    probs = logits / logits.sum(axis=-1, keepdims=True)
    order = np.argsort(-probs, axis=1, kind="stable")[:, :TOPK]

    onehot = np.zeros((T, E), dtype=bool)
    onehot[np.arange(T)[:, None], order] = True
    full_lists = [np.nonzero(onehot[:, e])[0] for e in range(E)]
    # Capacity dispatch: each core FFNs at most C_CAP tokens for its expert;
    # the small overflow (~1% of token-expert pairs at this routing) is
    # FFN'd on host in f32 below.
    tok_lists = [t[:C_CAP] for t in full_lists]
    over_lists = [t[C_CAP:] for t in full_lists]
    maxc = max(max(len(t) for t in tok_lists), N_T)
    C = int(-(-maxc // N_T) * N_T)  # round up to a multiple of N_T
    assert C <= C_CAP and C % N_T == 0 and C // N_T <= 512

    nc = _PROGRAM_CACHE.get(C)
    if nc is None:
        nc = _build_program(C)
        _PROGRAM_CACHE[C] = nc

    bf = ml_dtypes.bfloat16
    xf_bf = xf.astype(bf)

    def _gu_layout(w):  # [H, I] -> [p, i_tile, c, j]
        return np.ascontiguousarray(
            w.reshape(H // P, P, I // P, P).transpose(1, 2, 0, 3)
        )

    in_maps = []
    for e in range(E):
        idx = tok_lists[e]
        xe = np.zeros((C, Hh), dtype=bf)
        xe[: len(idx)] = xf_bf[idx]
        in_maps.append(
            {
                "xT": np.ascontiguousarray(xe.T),
                "wg": _gu_layout(wg[e].astype(bf)),
                "wu": _gu_layout(wu[e].astype(bf)),
                "wd": wd[e].astype(bf),
            }
        )

    from concourse.bass_utils import run_bass_kernel_spmd

    res = run_bass_kernel_spmd(nc, in_maps, core_ids=list(range(E)))
    LAST_RESULT = res

    out = np.zeros((T, Hh), dtype=np.float32)
    for e in range(E):
        idx = tok_lists[e]
        ye = np.asarray(res.results[e]["yT"]).T[: len(idx)]
        out[idx] += probs[idx, e][:, None] * ye.astype(np.float32)

    def _silu(v):
        return v / (1.0 + np.exp(-v))

    for e in range(E):
        idx = over_lists[e]
        if len(idx) == 0:
            continue
        xo = xf[idx]
        yo = (_silu(xo @ wg[e]) * (xo @ wu[e])) @ wd[e]
        out[idx] += probs[idx, e][:, None] * yo
    return out.reshape(B, S, Hh)

